# revision 3
# baseline (speedup 1.0000x reference)
"""Causal multi-head attention (B=2, H=16, S=2048, D=128) on 8 TRN2 NeuronCores.

Sharding: batch*heads (32) split across 8 cores, 4 heads per core.
Per-head algorithm (all on one core, fp16 matmuls / f32 accumulation):
  - load Q,K,V with dtype-cast DMA (f32 DRAM -> fp16 SBUF)
  - PE-transpose Q,K 128x128 tiles to get [d, s] layout (contraction on partitions)
  - S^T[k, q-block] = K^T.T @ Q^T per 128-k chunk, 512-wide q blocks, f32 PSUM
  - P^T = exp(S^T / sqrt(D)) on ScalarE (PSUM -> SBUF fp16); no max-subtraction
    needed (scores are ~N(0,1); exp of masked -10000 underflows to exactly 0 in
    the f64 reference as well, so hard zeros match it)
  - causal mask: skip fully-masked tiles; multiply the single diagonal tile by a
    0/1 lower-triangle mask after exp
  - O[q,:] = sum_k P^T[k,q].T @ V_aug[k,:] where V_aug has a ones column
    appended -> last output column is the softmax denominator
  - divide by the denominator (DVE reciprocal + tensor_scalar) and store f32
"""

import math

import numpy as np

import concourse.bass as bass
import concourse.tile as tile
from concourse import bacc, mybir
from concourse.bass_utils import run_bass_kernel_spmd

B, H, S, D = 2, 16, 2048, 128
N_CORES = 8
HPC = (B * H) // N_CORES  # heads per core
P = 128                   # partitions / head_dim / k-chunk
NT = S // P               # 16 k-chunks (and s-tiles) per head
QB = 512                  # q-block width (moving free dim of the score matmul)
NQB = S // QB             # 4 q-blocks per head
G = 2                     # score-psum group: G tiles of [128, 512] = G banks

FP16 = mybir.dt.float16
F32 = mybir.dt.float32
SCALE = 1.0 / math.sqrt(D)

_cache = {}


def _build_program():
    """Build (once) the single-core Bass/Tile program used SPMD on all cores."""
    if "nc" in _cache:
        return _cache["nc"]

    nc = bacc.Bacc("TRN2", target_bir_lowering=False, debug=False)

    q_d = nc.dram_tensor("q", [HPC * S, D], F32, kind="ExternalInput").ap()
    k_d = nc.dram_tensor("k", [HPC * S, D], F32, kind="ExternalInput").ap()
    v_d = nc.dram_tensor("v", [HPC * S, D], F32, kind="ExternalInput").ap()
    ident_d = nc.dram_tensor("ident", [P, P], FP16, kind="ExternalInput").ap()
    tri_d = nc.dram_tensor("tri", [P, P], FP16, kind="ExternalInput").ap()
    o_d = nc.dram_tensor("o", [HPC * S, D], F32, kind="ExternalOutput").ap()

    with tile.TileContext(nc) as tc:
        with (
            tc.tile_pool(name="consts", bufs=1) as consts,
            tc.tile_pool(name="qn", bufs=2) as qn_pool,
            tc.tile_pool(name="kn", bufs=2) as kn_pool,
            tc.tile_pool(name="qt", bufs=2) as qt_pool,
            tc.tile_pool(name="kt", bufs=2) as kt_pool,
            tc.tile_pool(name="vt", bufs=2) as vt_pool,
            tc.tile_pool(name="pt", bufs=2) as pt_pool,
            tc.tile_pool(name="ostage", bufs=2) as ostage_pool,
            tc.tile_pool(name="rec", bufs=4) as rec_pool,
            tc.tile_pool(name="stp", bufs=2, space="PSUM") as st_pool,
            tc.tile_pool(name="ops", bufs=2, space="PSUM") as o_pool,
            tc.tile_pool(name="tps", bufs=2, space="PSUM") as trans_pool,
        ):
            ident = consts.tile([P, P], FP16)
            nc.sync.dma_start(ident[:], ident_d[:])
            tri = consts.tile([P, P], FP16)
            nc.sync.dma_start(tri[:], tri_d[:])

            for h in range(HPC):
                rows = slice(h * S, (h + 1) * S)
                q_h = q_d[rows, :].rearrange("(n p) d -> p n d", p=P)
                k_h = k_d[rows, :].rearrange("(n p) d -> p n d", p=P)
                v_h = v_d[rows, :].rearrange("(n p) d -> p n d", p=P)
                o_h = o_d[rows, :].rearrange("(n p) d -> p n d", p=P)

                # ---- phase A: load (with cast) + transpose Q,K ----
                qn = qn_pool.tile([P, NT, P], FP16)
                nc.gpsimd.dma_start(qn[:], q_h)
                kn = kn_pool.tile([P, NT, P], FP16)
                nc.gpsimd.dma_start(kn[:], k_h)
                vt = vt_pool.tile([P, NT, P + 1], FP16)
                nc.gpsimd.dma_start(vt[:, :, 0:P], v_h)
                nc.vector.memset(vt[:, :, P : P + 1], 1.0)

                qt = qt_pool.tile([P, NT, P], FP16)
                kt = kt_pool.tile([P, NT, P], FP16)
                for n in range(NT):
                    tq = trans_pool.tile([P, P], FP16, tag="tp")
                    nc.tensor.transpose(tq[:], qn[:, n, :], ident[:])
                    nc.vector.tensor_copy(qt[:, n, :], tq[:])
                    tk = trans_pool.tile([P, P], FP16, tag="tp")
                    nc.tensor.transpose(tk[:], kn[:, n, :], ident[:])
                    nc.vector.tensor_copy(kt[:, n, :], tk[:])

                # ---- phase B: per q-block scores + softmax + PV ----
                ostage = ostage_pool.tile([P, NT, P], F32)
                for b in range(NQB):
                    nj = (b + 1) * (QB // P)  # valid k-chunks for this q block
                    qt_b = qt[:, b * (QB // P) : (b + 1) * (QB // P), :]
                    pt_b = pt_pool.tile([P, NT, QB], FP16)
                    for gs in range(0, nj, G):
                        gw = min(G, nj - gs)
                        stp = st_pool.tile([P, G, QB], F32)
                        for jj in range(gw):
                            nc.tensor.matmul(
                                stp[:, jj, :],
                                lhsT=kt[:, gs + jj, :],
                                rhs=qt_b,
                                start=True,
                                stop=True,
                            )
                        nc.scalar.activation(
                            pt_b[:, gs : gs + gw, :],
                            stp[:, 0:gw, :],
                            mybir.ActivationFunctionType.Exp,
                            scale=SCALE,
                        )
                        for jj in range(gw):
                            j = gs + jj
                            s_loc = j - 4 * b
                            if 0 <= s_loc < 4:
                                # diagonal tile: zero the strictly-upper part
                                dslc = pt_b[:, j, s_loc * P : (s_loc + 1) * P]
                                nc.vector.tensor_mul(dslc, dslc, tri[:])
                    for s_loc in range(4):
                        g = 4 * b + s_loc
                        o_ps = o_pool.tile([P, QB], F32)
                        for j in range(g + 1):
                            nc.tensor.matmul(
                                o_ps[:, 0 : P + 1],
                                lhsT=pt_b[:, j, s_loc * P : (s_loc + 1) * P],
                                rhs=vt[:, j, :],
                                start=(j == 0),
                                stop=(j == g),
                            )
                        rec = rec_pool.tile([P, 1], F32)
                        nc.vector.reciprocal(rec[:], o_ps[:, P : P + 1])
                        nc.vector.tensor_scalar_mul(
                            ostage[:, g, :], o_ps[:, 0:P], rec[:]
                        )
                nc.sync.dma_start(o_h, ostage[:])

    nc.compile()
    _cache["nc"] = nc
    return nc


def _make_const_inputs():
    ident = np.eye(P, dtype=np.float16)
    # tri[kk, qq] = 1 where qq >= kk (valid causal positions in S^T layout)
    tri = np.triu(np.ones((P, P), dtype=np.float16))
    return ident, tri


def run_sharded(q, k, v, trace=False, **kw):
    """q,k,v: [B,H,S,D] f32 -> (out [B,H,S,D] f32, BassKernelResults)."""
    nc = _build_program()
    qf = np.ascontiguousarray(np.asarray(q, dtype=np.float32).reshape(B * H, S, D))
    kf = np.ascontiguousarray(np.asarray(k, dtype=np.float32).reshape(B * H, S, D))
    vf = np.ascontiguousarray(np.asarray(v, dtype=np.float32).reshape(B * H, S, D))
    ident, tri = _make_const_inputs()
    in_maps = []
    for c in range(N_CORES):
        hs = slice(c * HPC, (c + 1) * HPC)
        in_maps.append(
            {
                "q": qf[hs].reshape(HPC * S, D),
                "k": kf[hs].reshape(HPC * S, D),
                "v": vf[hs].reshape(HPC * S, D),
                "ident": ident,
                "tri": tri,
            }
        )
    res = run_bass_kernel_spmd(nc, in_maps, list(range(N_CORES)), trace=trace, **kw)
    outs = [res.results[c]["o"].reshape(HPC, S, D) for c in range(N_CORES)]
    full = np.concatenate(outs, axis=0).reshape(B, H, S, D)
    return full, res


def kernel(query_states, key_states, value_states):
    out, _ = run_sharded(query_states, key_states, value_states)
    return out.astype(np.float32)
